# revision 14
# baseline (speedup 1.0000x reference)
"""Trainium2 Bass kernel for DiagonalUpsample (checkerboard 2x interleave).

  out[2i,   2j  ] = d[i,j];  out[2i,   2j+1] = u[i,j]
  out[2i+1, 2j  ] = u[i,j];  out[2i+1, 2j+1] = d[i,j]

Sharding: data parallel over flattened (b, c, h) input rows, but SKEWED
within each HBM-stack pair of NeuronCores: pnc pairs (2p, 2p+1) share one
HBM stack, and under full overlap the hardware arbitration consistently
favors the odd core (~56/44).  Balanced sharding therefore leaves the even
core as a straggler that finishes ~15-20us after its neighbor, capped by
the per-core DMA fabric once it is alone.  Splitting each pair's 24 work
tiles 11 (even) / 13 (odd) makes both cores finish together in every
observed regime (odd-favored contention / fair sharing / no overlap).

One shared NEFF handles both shard sizes: 13 tile slots of 256 input rows;
a tiny per-core `nwork` input (11 or 13) predicates the DMAs of slots 11
and 12 via cond= (skipped DMAs still bump semaphores, so the dependency
graph is identical on both cores).

Per-core layout: tile t, partition p holds local input rows t*256 + 2p +
(0..1) as 1024 contiguous f32; the matching 4 output rows are 4096
contiguous f32 -> loads and stores are fully contiguous HBM runs.  The
4-byte checkerboard interleave runs on the vector engine as 4 strided
tensor_copys per tile (fp32 2x_2P mode -- needs the row-count dim EVEN,
else DVE falls back to 1x).

DMA plan (all on the sync HWDGE ring, FIFO): one long read run (26 loads),
then one long write run (13 stores), so HBM never pays read/write
direction turnaround mid-kernel; the store ring-order pin is a free
same-ring FIFO edge, not a semaphore.  The output pool is 6 deep (~28us of
store-cadence cushion), so the interleave for store s+6 only waits on
store s's completion receipt and can never gate the ring -- with a shallow
pool, delayed receipts cascade into multi-us ring starvation gaps on
exactly the core that is already behind on arbitration.
"""

import numpy as np

import concourse.bass as bass
import concourse.tile as tile
from concourse import bacc, mybir
from concourse.bass_utils import run_bass_kernel_spmd
from concourse.tile import add_dep_helper

B, C, H, W = 16, 3, 512, 512
N_CORES = 8
ROWS_TOTAL = B * C * H         # 24576 input rows of 512 f32
P = 128                        # SBUF partitions
K = 1                          # input rows per partition per tile
TILE_ROWS = P * K              # 128 input rows per tile
PAIR_TILES = ROWS_TOTAL // TILE_ROWS // (N_CORES // 2)   # 48 tiles per pair
EVEN_TILES = 21                # tiles for the even (arbitration-losing) core
ODD_TILES = PAIR_TILES - EVEN_TILES  # 13 tiles for the odd core
N_SLOT = max(EVEN_TILES, ODD_TILES)  # NEFF tile slots
FP32 = mybir.dt.float32

_nc_cache = []

# test-harness knobs (ignored in normal grading use)
TRACE = False
LAST_RESULT = None


def _build_nc() -> bass.Bass:
    nc = bacc.Bacc("TRN2", debug=False)
    up = nc.dram_tensor("up", [N_SLOT, P, K * W], FP32, kind="ExternalInput")
    down = nc.dram_tensor("down", [N_SLOT, P, K * W], FP32, kind="ExternalInput")
    nwork = nc.dram_tensor("nwork", [1, 1], mybir.dt.uint32, kind="ExternalInput")
    out = nc.dram_tensor("out", [N_SLOT, P, K * 4 * W], FP32, kind="ExternalOutput")

    n_always = min(EVEN_TILES, ODD_TILES)

    with tile.TileContext(nc) as tc:
        with (
            tc.tile_pool(name="inp", bufs=N_SLOT) as inp,
            tc.tile_pool(name="outp", bufs=12) as outp,
        ):
            # number of valid tiles for this core, read once on the sync
            # sequencer (the engine that issues every DMA).  Loaded lazily,
            # right before the first conditional DMA, so the ~1us DRAM
            # reg_load latency hides behind the unconditional load
            # dispatches instead of delaying the first packet.
            nw_box = []

            def cond_for(t):
                if t < n_always:
                    return None
                if not nw_box:
                    nw_reg = nc.sync.alloc_register("nw_reg")
                    nc.sync.reg_load(nw_reg, nwork[0:1, 0:1])
                    nw_box.append(
                        nc.sync.snap(nw_reg, donate=True, min_val=0,
                                     max_val=N_SLOT)
                    )
                return nw_box[0] > t

            us, ds = [], []
            last_load = None
            for t in range(N_SLOT):
                cond = cond_for(t)
                ckw = {} if cond is None else {"cond": cond, "cond_hint": True}
                u = inp.tile([P, K * W], FP32, tag="u")
                nc.sync.dma_start(u[:], up[t], **ckw)
                d = inp.tile([P, K * W], FP32, tag="d")
                last_load = nc.sync.dma_start(d[:], down[t], **ckw)
                us.append(u)
                ds.append(d)
            for t in range(N_SLOT):
                o = outp.tile([P, K * 4 * W], FP32, tag="o")
                # per-partition layout: k (input row) x r (out-row
                # parity) x w (out col pair) x c (out col parity)
                ov = o.rearrange("p (k r w c) -> p k r c w", k=K, r=2, w=W, c=2)
                uv = us[t].rearrange("p (k w) -> p k w", k=K)
                dv = ds[t].rearrange("p (k w) -> p k w", k=K)
                nc.vector.tensor_copy(ov[:, :, 0, 0, :], dv[:])
                nc.vector.tensor_copy(ov[:, :, 0, 1, :], uv[:])
                nc.vector.tensor_copy(ov[:, :, 1, 0, :], uv[:])
                nc.vector.tensor_copy(ov[:, :, 1, 1, :], dv[:])
                sckw = ({} if cond_for(t) is None
                        else {"cond": cond_for(t), "cond_hint": True})
                store = nc.sync.dma_start(out[t], o[:], **sckw)
                # pin phase order: no store may be scheduled before the
                # read run completes (direction mixing costs HBM bw)
                add_dep_helper(store.ins, last_load.ins, sync=False,
                               reason="write phase after read phase")
    nc.compile()
    return nc


def _get_nc() -> bass.Bass:
    if not _nc_cache:
        _nc_cache.append(_build_nc())
    return _nc_cache[0]


def _core_tiles(core: int) -> int:
    return EVEN_TILES if core % 2 == 0 else ODD_TILES


def kernel(up_diagonal: np.ndarray, down_diagonal: np.ndarray) -> np.ndarray:
    up_diagonal = np.ascontiguousarray(np.asarray(up_diagonal, dtype=np.float32))
    down_diagonal = np.ascontiguousarray(np.asarray(down_diagonal, dtype=np.float32))
    assert up_diagonal.shape == (B, C, H, W), up_diagonal.shape

    up_rows = up_diagonal.reshape(ROWS_TOTAL, W)
    down_rows = down_diagonal.reshape(ROWS_TOTAL, W)

    nc = _get_nc()
    in_maps = []
    row0 = 0
    bounds = []
    for core in range(N_CORES):
        nt = _core_tiles(core)
        rows = nt * TILE_ROWS
        bounds.append((row0, row0 + rows))
        u = np.zeros((N_SLOT, P, K * W), dtype=np.float32)
        d = np.zeros((N_SLOT, P, K * W), dtype=np.float32)
        u[:nt] = up_rows[row0:row0 + rows].reshape(nt, P, K * W)
        d[:nt] = down_rows[row0:row0 + rows].reshape(nt, P, K * W)
        in_maps.append(
            {"up": u, "down": d,
             "nwork": np.array([[nt]], dtype=np.uint32)}
        )
        row0 += rows
    assert row0 == ROWS_TOTAL

    res = run_bass_kernel_spmd(
        nc, in_maps, core_ids=list(range(N_CORES)), trace=TRACE
    )
    global LAST_RESULT
    LAST_RESULT = res
    results = res.results
    out_rows = np.empty((2 * ROWS_TOTAL, 2 * W), dtype=np.float32)
    for core in range(N_CORES):
        nt = _core_tiles(core)
        a, b = bounds[core]
        out_rows[2 * a:2 * b] = results[core]["out"][:nt].reshape(-1, 2 * W)
    return out_rows.reshape(B, C, 2 * H, 2 * W)


# revision 15
# speedup vs baseline: 1.0155x; 1.0155x over previous
"""Trainium2 Bass kernel for DiagonalUpsample (checkerboard 2x interleave).

  out[2i,   2j  ] = d[i,j];  out[2i,   2j+1] = u[i,j]
  out[2i+1, 2j  ] = u[i,j];  out[2i+1, 2j+1] = d[i,j]

Sharding: data parallel over flattened (b, c, h) input rows, but SKEWED
within each HBM-stack pair of NeuronCores: pnc pairs (2p, 2p+1) share one
HBM stack, and under full overlap the hardware arbitration consistently
favors the odd core (~56/44).  Balanced sharding therefore leaves the even
core as a straggler that finishes ~15-20us after its neighbor, capped by
the per-core DMA fabric once it is alone.  Splitting each pair's 24 work
tiles 11 (even) / 13 (odd) makes both cores finish together in every
observed regime (odd-favored contention / fair sharing / no overlap).

One shared NEFF handles both shard sizes: 13 tile slots of 256 input rows;
a tiny per-core `nwork` input (11 or 13) predicates the DMAs of slots 11
and 12 via cond= (skipped DMAs still bump semaphores, so the dependency
graph is identical on both cores).

Per-core layout: tile t, partition p holds local input rows t*256 + 2p +
(0..1) as 1024 contiguous f32; the matching 4 output rows are 4096
contiguous f32 -> loads and stores are fully contiguous HBM runs.  The
4-byte checkerboard interleave runs on the vector engine as 4 strided
tensor_copys per tile (fp32 2x_2P mode -- needs the row-count dim EVEN,
else DVE falls back to 1x).

DMA plan (all on the sync HWDGE ring, FIFO): one long read run (26 loads),
then one long write run (13 stores), so HBM never pays read/write
direction turnaround mid-kernel; the store ring-order pin is a free
same-ring FIFO edge, not a semaphore.  The output pool is 6 deep (~28us of
store-cadence cushion), so the interleave for store s+6 only waits on
store s's completion receipt and can never gate the ring -- with a shallow
pool, delayed receipts cascade into multi-us ring starvation gaps on
exactly the core that is already behind on arbitration.
"""

import numpy as np

import concourse.bass as bass
import concourse.tile as tile
from concourse import bacc, mybir
from concourse.bass_utils import run_bass_kernel_spmd
from concourse.tile import add_dep_helper

B, C, H, W = 16, 3, 512, 512
N_CORES = 8
ROWS_TOTAL = B * C * H         # 24576 input rows of 512 f32
P = 128                        # SBUF partitions
K = 2                          # input rows per partition per tile
TILE_ROWS = P * K              # 256 input rows per tile
PAIR_TILES = ROWS_TOTAL // TILE_ROWS // (N_CORES // 2)   # 24 tiles per pair
EVEN_TILES = 11                # tiles for the even (arbitration-losing) core
ODD_TILES = PAIR_TILES - EVEN_TILES  # 13 tiles for the odd core
N_SLOT = max(EVEN_TILES, ODD_TILES)  # NEFF tile slots
FP32 = mybir.dt.float32

_nc_cache = []

# test-harness knobs (ignored in normal grading use)
TRACE = False
LAST_RESULT = None


def _build_nc() -> bass.Bass:
    nc = bacc.Bacc("TRN2", debug=False)
    up = nc.dram_tensor("up", [N_SLOT, P, K * W], FP32, kind="ExternalInput")
    down = nc.dram_tensor("down", [N_SLOT, P, K * W], FP32, kind="ExternalInput")
    nwork = nc.dram_tensor("nwork", [1, 1], mybir.dt.uint32, kind="ExternalInput")
    out = nc.dram_tensor("out", [N_SLOT, P, K * 4 * W], FP32, kind="ExternalOutput")

    n_always = min(EVEN_TILES, ODD_TILES)

    with tile.TileContext(nc) as tc:
        with (
            tc.tile_pool(name="inp", bufs=N_SLOT) as inp,
            tc.tile_pool(name="outp", bufs=6) as outp,
        ):
            # number of valid tiles for this core, read once on the sync
            # sequencer (the engine that issues every DMA).  Loaded lazily,
            # right before the first conditional DMA, so the ~1us DRAM
            # reg_load latency hides behind the unconditional load
            # dispatches instead of delaying the first packet.
            nw_box = []

            def cond_for(t):
                if t < n_always:
                    return None
                if not nw_box:
                    nw_reg = nc.sync.alloc_register("nw_reg")
                    nc.sync.reg_load(nw_reg, nwork[0:1, 0:1])
                    nw_box.append(
                        nc.sync.snap(nw_reg, donate=True, min_val=0,
                                     max_val=N_SLOT)
                    )
                return nw_box[0] > t

            us, ds = [], []
            last_load = None
            for t in range(N_SLOT):
                cond = cond_for(t)
                ckw = {} if cond is None else {"cond": cond, "cond_hint": True}
                u = inp.tile([P, K * W], FP32, tag="u")
                nc.sync.dma_start(u[:], up[t], **ckw)
                d = inp.tile([P, K * W], FP32, tag="d")
                last_load = nc.sync.dma_start(d[:], down[t], **ckw)
                us.append(u)
                ds.append(d)
            for t in range(N_SLOT):
                o = outp.tile([P, K * 4 * W], FP32, tag="o")
                # per-partition layout: k (input row) x r (out-row
                # parity) x w (out col pair) x c (out col parity)
                ov = o.rearrange("p (k r w c) -> p k r c w", k=K, r=2, w=W, c=2)
                uv = us[t].rearrange("p (k w) -> p k w", k=K)
                dv = ds[t].rearrange("p (k w) -> p k w", k=K)
                nc.vector.tensor_copy(ov[:, :, 0, 0, :], dv[:])
                nc.vector.tensor_copy(ov[:, :, 0, 1, :], uv[:])
                nc.vector.tensor_copy(ov[:, :, 1, 0, :], uv[:])
                nc.vector.tensor_copy(ov[:, :, 1, 1, :], dv[:])
                sckw = ({} if cond_for(t) is None
                        else {"cond": cond_for(t), "cond_hint": True})
                store = nc.sync.dma_start(out[t], o[:], **sckw)
                # pin phase order: no store may be scheduled before the
                # read run completes (direction mixing costs HBM bw)
                add_dep_helper(store.ins, last_load.ins, sync=False,
                               reason="write phase after read phase")
    nc.compile()
    return nc


def _get_nc() -> bass.Bass:
    if not _nc_cache:
        _nc_cache.append(_build_nc())
    return _nc_cache[0]


def _core_tiles(core: int) -> int:
    return EVEN_TILES if core % 2 == 0 else ODD_TILES


def kernel(up_diagonal: np.ndarray, down_diagonal: np.ndarray) -> np.ndarray:
    up_diagonal = np.ascontiguousarray(np.asarray(up_diagonal, dtype=np.float32))
    down_diagonal = np.ascontiguousarray(np.asarray(down_diagonal, dtype=np.float32))
    assert up_diagonal.shape == (B, C, H, W), up_diagonal.shape

    up_rows = up_diagonal.reshape(ROWS_TOTAL, W)
    down_rows = down_diagonal.reshape(ROWS_TOTAL, W)

    nc = _get_nc()
    in_maps = []
    row0 = 0
    bounds = []
    for core in range(N_CORES):
        nt = _core_tiles(core)
        rows = nt * TILE_ROWS
        bounds.append((row0, row0 + rows))
        u = np.zeros((N_SLOT, P, K * W), dtype=np.float32)
        d = np.zeros((N_SLOT, P, K * W), dtype=np.float32)
        u[:nt] = up_rows[row0:row0 + rows].reshape(nt, P, K * W)
        d[:nt] = down_rows[row0:row0 + rows].reshape(nt, P, K * W)
        in_maps.append(
            {"up": u, "down": d,
             "nwork": np.array([[nt]], dtype=np.uint32)}
        )
        row0 += rows
    assert row0 == ROWS_TOTAL

    res = run_bass_kernel_spmd(
        nc, in_maps, core_ids=list(range(N_CORES)), trace=TRACE
    )
    global LAST_RESULT
    LAST_RESULT = res
    results = res.results
    out_rows = np.empty((2 * ROWS_TOTAL, 2 * W), dtype=np.float32)
    for core in range(N_CORES):
        nt = _core_tiles(core)
        a, b = bounds[core]
        out_rows[2 * a:2 * b] = results[core]["out"][:nt].reshape(-1, 2 * W)
    return out_rows.reshape(B, C, 2 * H, 2 * W)
